# revision 7
# baseline (speedup 1.0000x reference)
# Trainium2 Bass kernel for nn_MultiCondLayer:
#   out[b,o,n] = (sum_k (cond[b] @ W[k].T)[o,n] + sum_k b[k,o]) * x_mask[b,0,n]
# Algebraic reduction: sum_k Linear_k(x) == Linear(x) with W' = sum_k W[k],
# b' = sum_k b[k]  (4x FLOP reduction vs. the naive einsum over k).
#
# Sharding: data-parallel over batch B=8 across the 8 NeuronCores (one batch
# element per core); the reduced [1024,1024] weight is replicated.
#
# Precision: all operands are cast to bf16 on the host (x, W', mask) and the
# output is stored bf16 and upcast on the host. PSUM accumulation stays fp32.
# The PE streams bf16 at the same 1 col/cycle as fp32r, so this does not
# change the ~110us matmul floor, but it (a) halves HBM traffic 38->19 MB
# per core, (b) enables FWL so LDWEIGHTS (~330ns in fp32) fully hides, and
# (c) halves the startup ramp and store tail. End-to-end rel err ~2e-3,
# well under the 2e-2 gate.
#
# Schedule: at body start the PE warms the HAM clock gate with 8 dummy
# matmuls on memset data (no DMA dependency), then broadcasts the mask row
# across partitions via ones-outer-product (real work that keeps warming).
# DMA queues: mask row on the otherwise-idle gpsimd SWDGE queue; x window
# chunks on the sync HWDGE queue; weights (o-halved), bias, and out-stores
# on the scalar HWDGE queue. Main stream: for each 1024-wide n-window and
# each o-tile, one serial c-chain of 8x(LDW + 2 matmuls) accumulating a
# 2-bank psum pair, evicted by fused DVE (psum+bias)*mask into a bf16
# [128,1024] out tile and stored. Only 2 psum banks per chain are in
# flight, so evictions stagger and bank reuse has ~4 chains of slack.

import numpy as np

import ml_dtypes

import concourse.bass as bass
import concourse.mybir as mybir
import concourse.tile as tile
from concourse import bacc
from concourse.bass_utils import run_bass_kernel_spmd

P = 128
B, C, N = 8, 1024, 4096
O = 1024
NT = 512                 # matmul free dim = one fp32 PSUM bank
CO, OO = C // P, O // P
# n-window plan: narrow first window so the startup-critical x DMA is only
# 1MB (PE ramp covers it); narrow last window so the final evict+store tail
# is one bank / 128KB.
WINDOWS = [512, 1024, 1024, 1024, 512]
F32 = mybir.dt.float32
BF16 = mybir.dt.bfloat16

N_CORES = 8
N_WARM = 8               # dummy matmuls to warm the HAM clock gate


def build_module():
    nc = bacc.Bacc("TRN2", target_bir_lowering=False, debug=False,
                   num_devices=N_CORES)
    x = nc.dram_tensor("x", [C, N], BF16, kind="ExternalInput")    # cond[b]
    wt = nc.dram_tensor("wt", [C, O], BF16, kind="ExternalInput")  # (sum_k W[k]).T
    # bias pre-transposed on host to [128, OO]: 128 contiguous rows.
    bv = nc.dram_tensor("bv", [P, OO], F32, kind="ExternalInput")
    mk = nc.dram_tensor("mk", [N], BF16, kind="ExternalInput")     # x_mask[b,0]
    out = nc.dram_tensor("out", [O, N], BF16, kind="ExternalOutput")

    x_r = x.ap().rearrange("(c p) n -> p c n", p=P)      # [128, CO, N]
    wt_r = wt.ap().rearrange("(c p) o -> p c o", p=P)    # [128, CO, O]

    with tile.TileContext(nc) as tc:
        with (
            tc.tile_pool(name="consts", bufs=1) as consts,
            tc.tile_pool(name="outs", bufs=6) as outs,
            tc.tile_pool(name="ps", bufs=8, space="PSUM") as psp,
        ):
            # --- DMA issue order matters: small things first per queue. ---
            # gpsimd (SWDGE, otherwise idle): the 8KB mask row + 4KB bias.
            mkrow_sb = consts.tile([1, N], BF16)
            nc.gpsimd.dma_start(mkrow_sb[:], mk.ap()[None, :])
            bias_sb = consts.tile([P, OO], F32)
            nc.gpsimd.dma_start(bias_sb[:], bv.ap())
            # scalar HWDGE: weights in two 1MB halves (one DMA instruction
            # each -- HWDGE issue costs ~0.6us per dma_start, so few big
            # DMAs beat many small ones). Chains walk o serially, so the
            # upper half has ~7us of slack.
            OH = O // 2
            w_sb = consts.tile([P, CO, O], BF16)
            nc.scalar.dma_start(w_sb[:, :, 0:OH], wt_r[:, :, 0:OH])
            nc.scalar.dma_start(w_sb[:, :, OH:O], wt_r[:, :, OH:O])
            # sync HWDGE: x as one DMA per n-window, fully resident.
            x_sb = consts.tile([P, CO, N], BF16)
            n0 = 0
            for nw in WINDOWS:
                nc.sync.dma_start(x_sb[:, :, n0:n0 + nw],
                                  x_r[:, :, n0:n0 + nw])
                n0 += nw

            # --- PE warmup: no-DMA dummy matmuls release the HAM throttle
            # (cold 1.2GHz -> warm 2.4GHz needs ~3.4us of sustained busy)
            # while the first real chunks are still in flight. ---
            scratch = consts.tile([P, NT], BF16)
            nc.vector.memset(scratch[:], 0.0)
            ones_sb = consts.tile([1, P], BF16)
            nc.vector.memset(ones_sb[:], 1.0)
            for i in range(N_WARM):
                wps = psp.tile([P, NT], F32, name=f"warm_{i}", tag="ps")
                nc.tensor.matmul(wps[:], scratch[:, 0:P], scratch[:],
                                 start=True, stop=True)

            # --- Mask broadcast on-chip: ones[128,1] (x) mkrow[1,N] via PE
            # (keeps warming; avoids a 128x replicated mask DMA). ---
            mask_sb = consts.tile([P, N], BF16)
            for n in range(N // NT):
                mps = psp.tile([P, NT], F32, name=f"mps_{n}", tag="ps")
                nc.tensor.matmul(mps[:], ones_sb[:],
                                 mkrow_sb[:, n * NT:(n + 1) * NT],
                                 start=True, stop=True)
                nc.vector.tensor_copy(mask_sb[:, n * NT:(n + 1) * NT], mps[:])

            # --- Main stream: per n-window, 8 serial o-chains of 8 c-steps;
            # 1024-wide windows use a 2-bank psum pair per chain, 512-wide
            # use a single bank. Only 1-2 banks in flight per chain, so
            # evictions stagger and bank reuse has ~4 chains of slack. ---
            n0 = 0
            for ns, nw in enumerate(WINDOWS):
                nsub = nw // NT
                for o in range(OO):
                    pss = [psp.tile([P, NT], F32, name=f"ps_{ns}_{o}_{j}",
                                    tag="ps") for j in range(nsub)]
                    for c in range(CO):
                        w_ap = w_sb[:, c, o * P:(o + 1) * P]
                        for j in range(nsub):
                            nj = n0 + j * NT
                            nc.tensor.matmul(pss[j][:], w_ap,
                                             x_sb[:, c, nj:nj + NT],
                                             start=(c == 0),
                                             stop=(c == CO - 1))
                    ot = outs.tile([P, nw], BF16, name=f"ot_{ns}_{o}",
                                   tag=f"ot{nw}")
                    for j in range(nsub):
                        nj = n0 + j * NT
                        nc.vector.scalar_tensor_tensor(
                            ot[:, j * NT:(j + 1) * NT], pss[j][:],
                            bias_sb[:, o:o + 1], mask_sb[:, nj:nj + NT],
                            op0=mybir.AluOpType.add,
                            op1=mybir.AluOpType.mult)
                    nc.scalar.dma_start(
                        out.ap()[o * P:(o + 1) * P, n0:n0 + nw], ot[:])
                n0 += nw
    nc.compile()
    return nc


_NC_CACHE = None


def _get_module():
    global _NC_CACHE
    if _NC_CACHE is None:
        _NC_CACHE = build_module()
    return _NC_CACHE


def _make_in_maps(cond, x_mask, W, b):
    bf16 = ml_dtypes.bfloat16
    wt = np.ascontiguousarray(
        W.astype(np.float32).sum(axis=0).T.astype(bf16))           # [C, O]
    bv = np.ascontiguousarray(
        b.astype(np.float32).sum(axis=0).reshape(OO, P).T,
        dtype=np.float32)                                          # [128, OO]
    in_maps = []
    for core in range(N_CORES):
        in_maps.append({
            "x": np.ascontiguousarray(cond[core].astype(bf16)),
            "wt": wt,
            "bv": bv,
            "mk": np.ascontiguousarray(x_mask[core, 0].astype(bf16)),
        })
    return in_maps


def run(cond, x_mask, W, b, trace=False, trace_cores=None):
    """Run on hardware; returns (out [B,O,N] fp32, BassKernelResults)."""
    nc = _get_module()
    in_maps = _make_in_maps(cond, x_mask, W, b)
    res = run_bass_kernel_spmd(
        nc, in_maps, core_ids=list(range(N_CORES)),
        trace=trace, trace_cores=trace_cores,
    )
    out = np.stack(
        [res.results[i]["out"].astype(np.float32) for i in range(N_CORES)],
        axis=0)
    return out, res


def kernel(cond, x_mask, W, b):
    out, _ = run(cond, x_mask, W, b)
    return out


# revision 8
# speedup vs baseline: 1.0215x; 1.0215x over previous
# Trainium2 Bass kernel for nn_MultiCondLayer:
#   out[b,o,n] = (sum_k (cond[b] @ W[k].T)[o,n] + sum_k b[k,o]) * x_mask[b,0,n]
# Algebraic reduction: sum_k Linear_k(x) == Linear(x) with W' = sum_k W[k],
# b' = sum_k b[k]  (4x FLOP reduction vs. the naive einsum over k).
#
# Sharding: data-parallel over batch B=8 across the 8 NeuronCores (one batch
# element per core); the reduced [1024,1024] weight is replicated.
#
# Precision: all operands are cast to bf16 on the host (x, W', mask) and the
# output is stored bf16 and upcast on the host. PSUM accumulation stays fp32.
# The PE streams bf16 at the same 1 col/cycle as fp32r, so this does not
# change the ~110us matmul floor, but it (a) halves HBM traffic 38->19 MB
# per core, (b) enables FWL so LDWEIGHTS (~330ns in fp32) fully hides, and
# (c) halves the startup ramp and store tail. End-to-end rel err ~2e-3,
# well under the 2e-2 gate.
#
# Schedule: at body start the PE warms the HAM clock gate with 8 dummy
# matmuls on memset data (no DMA dependency), then broadcasts the mask row
# across partitions via ones-outer-product (real work that keeps warming).
# DMA queues: mask row on the otherwise-idle gpsimd SWDGE queue; x window
# chunks on the sync HWDGE queue; weights (o-halved), bias, and out-stores
# on the scalar HWDGE queue. Main stream: for each 1024-wide n-window and
# each o-tile, one serial c-chain of 8x(LDW + 2 matmuls) accumulating a
# 2-bank psum pair, evicted by fused DVE (psum+bias)*mask into a bf16
# [128,1024] out tile and stored. Only 2 psum banks per chain are in
# flight, so evictions stagger and bank reuse has ~4 chains of slack.

import numpy as np

import ml_dtypes

import concourse.bass as bass
import concourse.mybir as mybir
import concourse.tile as tile
from concourse import bacc
from concourse.bass_utils import run_bass_kernel_spmd

P = 128
B, C, N = 8, 1024, 4096
O = 1024
NT = 512                 # matmul free dim = one fp32 PSUM bank
CO, OO = C // P, O // P
# n-window plan: narrow first window so the startup-critical x DMA is only
# 1MB (PE ramp covers it); narrow last window so the final evict+store tail
# is one bank / 128KB.
WINDOWS = [512, 1024, 1024, 1024, 512]
F32 = mybir.dt.float32
BF16 = mybir.dt.bfloat16

N_CORES = 8
N_WARM = 8               # dummy matmuls to warm the HAM clock gate


def build_module():
    nc = bacc.Bacc("TRN2", target_bir_lowering=False, debug=False,
                   num_devices=N_CORES)
    x = nc.dram_tensor("x", [C, N], BF16, kind="ExternalInput")    # cond[b]
    wt = nc.dram_tensor("wt", [C, O], BF16, kind="ExternalInput")  # (sum_k W[k]).T
    # bias pre-transposed on host to [128, OO]: 128 contiguous rows.
    bv = nc.dram_tensor("bv", [P, OO], F32, kind="ExternalInput")
    mk = nc.dram_tensor("mk", [N], BF16, kind="ExternalInput")     # x_mask[b,0]
    out = nc.dram_tensor("out", [O, N], BF16, kind="ExternalOutput")

    x_r = x.ap().rearrange("(c p) n -> p c n", p=P)      # [128, CO, N]
    wt_r = wt.ap().rearrange("(c p) o -> p c o", p=P)    # [128, CO, O]

    with tile.TileContext(nc) as tc:
        with (
            tc.tile_pool(name="consts", bufs=1) as consts,
            tc.tile_pool(name="outs", bufs=6) as outs,
            tc.tile_pool(name="ps", bufs=8, space="PSUM") as psp,
        ):
            # --- DMA issue order matters: each HWDGE ring is FIFO, so the
            # ring order IS the priority order, and rings share the ~358
            # GB/s HBM limit round-robin. Critical first: the tiny mask row
            # and the first x window + first weight half must land by
            # ~13us; the bulk drains behind them. ---
            # sync ring: mask row, then x windows in consumption order
            # (one big DMA per window -- HWDGE issue costs ~0.6us each).
            mkrow_sb = consts.tile([1, N], BF16)
            nc.sync.dma_start(mkrow_sb[:], mk.ap()[None, :])
            x_sb = consts.tile([P, CO, N], BF16)
            n0 = 0
            for nw in WINDOWS:
                nc.sync.dma_start(x_sb[:, :, n0:n0 + nw],
                                  x_r[:, :, n0:n0 + nw])
                n0 += nw
            # scalar ring: weights in two 1MB halves (chains walk o
            # serially, so the upper half has ~7us of slack); out-stores
            # ride behind.
            OH = O // 2
            w_sb = consts.tile([P, CO, O], BF16)
            nc.scalar.dma_start(w_sb[:, :, 0:OH], wt_r[:, :, 0:OH])
            nc.scalar.dma_start(w_sb[:, :, OH:O], wt_r[:, :, OH:O])

            # --- PE warmup: no-DMA dummy matmuls release the HAM throttle
            # (cold 1.2GHz -> warm 2.4GHz needs ~3.4us of UNBROKEN busy)
            # while the first real chunks are still in flight. Memsets on
            # gpsimd, whose queue is otherwise slack, to start earliest. ---
            scratch = consts.tile([P, NT], BF16)
            nc.gpsimd.memset(scratch[:], 0.0)
            ones_sb = consts.tile([1, P], BF16)
            nc.gpsimd.memset(ones_sb[:], 1.0)
            bias_sb = consts.tile([P, OO], F32)
            nc.gpsimd.dma_start(bias_sb[:], bv.ap())
            for i in range(N_WARM):
                wps = psp.tile([P, NT], F32, name=f"warm_{i}", tag="ps")
                nc.tensor.matmul(wps[:], scratch[:, 0:P], scratch[:],
                                 start=True, stop=True)

            # --- Mask broadcast on-chip: ones[128,1] (x) mkrow[1,N] via PE
            # (keeps warming; avoids a 128x replicated mask DMA). ---
            mask_sb = consts.tile([P, N], BF16)
            for n in range(N // NT):
                mps = psp.tile([P, NT], F32, name=f"mps_{n}", tag="ps")
                nc.tensor.matmul(mps[:], ones_sb[:],
                                 mkrow_sb[:, n * NT:(n + 1) * NT],
                                 start=True, stop=True)
                nc.vector.tensor_copy(mask_sb[:, n * NT:(n + 1) * NT], mps[:])

            # --- Main stream: per n-window, 8 serial o-chains of 8 c-steps;
            # 1024-wide windows use a 2-bank psum pair per chain, 512-wide
            # use a single bank. Only 1-2 banks in flight per chain, so
            # evictions stagger and bank reuse has ~4 chains of slack. ---
            n0 = 0
            for ns, nw in enumerate(WINDOWS):
                nsub = nw // NT
                for o in range(OO):
                    pss = [psp.tile([P, NT], F32, name=f"ps_{ns}_{o}_{j}",
                                    tag="ps") for j in range(nsub)]
                    for c in range(CO):
                        w_ap = w_sb[:, c, o * P:(o + 1) * P]
                        for j in range(nsub):
                            nj = n0 + j * NT
                            nc.tensor.matmul(pss[j][:], w_ap,
                                             x_sb[:, c, nj:nj + NT],
                                             start=(c == 0),
                                             stop=(c == CO - 1))
                    ot = outs.tile([P, nw], BF16, name=f"ot_{ns}_{o}",
                                   tag=f"ot{nw}")
                    for j in range(nsub):
                        nj = n0 + j * NT
                        nc.vector.scalar_tensor_tensor(
                            ot[:, j * NT:(j + 1) * NT], pss[j][:],
                            bias_sb[:, o:o + 1], mask_sb[:, nj:nj + NT],
                            op0=mybir.AluOpType.add,
                            op1=mybir.AluOpType.mult)
                    nc.scalar.dma_start(
                        out.ap()[o * P:(o + 1) * P, n0:n0 + nw], ot[:])
                n0 += nw
    nc.compile()
    return nc


_NC_CACHE = None


def _get_module():
    global _NC_CACHE
    if _NC_CACHE is None:
        _NC_CACHE = build_module()
    return _NC_CACHE


def _make_in_maps(cond, x_mask, W, b):
    bf16 = ml_dtypes.bfloat16
    wt = np.ascontiguousarray(
        W.astype(np.float32).sum(axis=0).T.astype(bf16))           # [C, O]
    bv = np.ascontiguousarray(
        b.astype(np.float32).sum(axis=0).reshape(OO, P).T,
        dtype=np.float32)                                          # [128, OO]
    in_maps = []
    for core in range(N_CORES):
        in_maps.append({
            "x": np.ascontiguousarray(cond[core].astype(bf16)),
            "wt": wt,
            "bv": bv,
            "mk": np.ascontiguousarray(x_mask[core, 0].astype(bf16)),
        })
    return in_maps


def run(cond, x_mask, W, b, trace=False, trace_cores=None):
    """Run on hardware; returns (out [B,O,N] fp32, BassKernelResults)."""
    nc = _get_module()
    in_maps = _make_in_maps(cond, x_mask, W, b)
    res = run_bass_kernel_spmd(
        nc, in_maps, core_ids=list(range(N_CORES)),
        trace=trace, trace_cores=trace_cores,
    )
    out = np.stack(
        [res.results[i]["out"].astype(np.float32) for i in range(N_CORES)],
        axis=0)
    return out, res


def kernel(cond, x_mask, W, b):
    out, _ = run(cond, x_mask, W, b)
    return out


# revision 9
# speedup vs baseline: 1.0354x; 1.0136x over previous
# Trainium2 Bass kernel for nn_MultiCondLayer:
#   out[b,o,n] = (sum_k (cond[b] @ W[k].T)[o,n] + sum_k b[k,o]) * x_mask[b,0,n]
# Algebraic reduction: sum_k Linear_k(x) == Linear(x) with W' = sum_k W[k],
# b' = sum_k b[k]  (4x FLOP reduction vs. the naive einsum over k).
#
# Sharding: data-parallel over batch B=8 across the 8 NeuronCores (one batch
# element per core); the reduced [1024,1024] weight is replicated.
#
# Precision: all operands are cast to bf16 on the host (x, W', mask) and the
# output is stored bf16 and upcast on the host. PSUM accumulation stays fp32.
# The PE streams bf16 at the same 1 col/cycle as fp32r, so this does not
# change the ~110us matmul floor, but it (a) halves HBM traffic 38->19 MB
# per core, (b) enables FWL so LDWEIGHTS (~330ns in fp32) fully hides, and
# (c) halves the startup ramp and store tail. End-to-end rel err ~2e-3,
# well under the 2e-2 gate.
#
# Schedule: at body start the PE warms the HAM clock gate with 8 dummy
# matmuls on memset data (no DMA dependency), then broadcasts the mask row
# across partitions via ones-outer-product (real work that keeps warming).
# DMA queues: mask row on the otherwise-idle gpsimd SWDGE queue; x window
# chunks on the sync HWDGE queue; weights (o-halved), bias, and out-stores
# on the scalar HWDGE queue. Main stream: for each 1024-wide n-window and
# each o-tile, one serial c-chain of 8x(LDW + 2 matmuls) accumulating a
# 2-bank psum pair, evicted by fused DVE (psum+bias)*mask into a bf16
# [128,1024] out tile and stored. Only 2 psum banks per chain are in
# flight, so evictions stagger and bank reuse has ~4 chains of slack.

import numpy as np

import ml_dtypes

import concourse.bass as bass
import concourse.mybir as mybir
import concourse.tile as tile
from concourse import bacc
from concourse.bass_utils import run_bass_kernel_spmd

P = 128
B, C, N = 8, 1024, 4096
O = 1024
NT = 512                 # matmul free dim = one fp32 PSUM bank
CO, OO = C // P, O // P
# n-window plan: narrow first window so the startup-critical x DMA is only
# 1MB (PE ramp covers it); narrow last window so the final evict+store tail
# is one bank / 128KB.
WINDOWS = [512, 1024, 1024, 1024, 512]
F32 = mybir.dt.float32
BF16 = mybir.dt.bfloat16

N_CORES = 8
N_WARM = 8               # dummy matmuls to warm the HAM clock gate


def build_module():
    nc = bacc.Bacc("TRN2", target_bir_lowering=False, debug=False,
                   num_devices=N_CORES)
    x = nc.dram_tensor("x", [C, N], BF16, kind="ExternalInput")    # cond[b]
    wt = nc.dram_tensor("wt", [C, O], BF16, kind="ExternalInput")  # (sum_k W[k]).T
    # bias pre-transposed on host to [128, OO]: 128 contiguous rows.
    bv = nc.dram_tensor("bv", [P, OO], F32, kind="ExternalInput")
    mk = nc.dram_tensor("mk", [N], BF16, kind="ExternalInput")     # x_mask[b,0]
    out = nc.dram_tensor("out", [O, N], BF16, kind="ExternalOutput")

    x_r = x.ap().rearrange("(c p) n -> p c n", p=P)      # [128, CO, N]
    wt_r = wt.ap().rearrange("(c p) o -> p c o", p=P)    # [128, CO, O]

    with tile.TileContext(nc) as tc:
        with (
            tc.tile_pool(name="consts", bufs=1) as consts,
            tc.tile_pool(name="outs", bufs=6) as outs,
            tc.tile_pool(name="ps", bufs=8, space="PSUM") as psp,
        ):
            # --- DMA plan. All in-flight DMAs share the ~358 GB/s HBM
            # limit round-robin regardless of ring, so the only pacing
            # levers are issue order, chunk size (issue costs ~0.6us per
            # dma_start) and Tile's outstanding-DMA limiter. Keep the
            # startup-critical set small: mask row (8KB), x window 0
            # (1MB), first w o-pair column (512KB). ---
            # gpsimd: warmup memsets first (they gate the PE ramp), then
            # the mask row and bias on the otherwise-idle SWDGE queue.
            scratch = consts.tile([P, NT], BF16)
            nc.gpsimd.memset(scratch[:], 0.0)
            ones_sb = consts.tile([1, P], BF16)
            nc.gpsimd.memset(ones_sb[:], 1.0)
            mkrow_sb = consts.tile([1, N], BF16)
            nc.gpsimd.dma_start(mkrow_sb[:], mk.ap()[None, :])
            bias_sb = consts.tile([P, OO], F32)
            nc.gpsimd.dma_start(bias_sb[:], bv.ap())
            # sync ring: x in per-(window, c) chunks in consumption order;
            # per-chunk issue cost + the outstanding limiter pace the bulk
            # so it cannot starve the critical head.
            x_sb = consts.tile([P, CO, N], BF16)
            n0 = 0
            for nw in WINDOWS:
                for c in range(CO):
                    nc.sync.dma_start(x_sb[:, c, n0:n0 + nw],
                                      x_r[:, c, n0:n0 + nw])
                n0 += nw
            # scalar ring: w in o-pair column chunks (chains walk o
            # serially, so chain o0/o1 only needs the first 512KB column);
            # out-stores ride behind.
            OP = 2 * P
            w_sb = consts.tile([P, CO, O], BF16)
            for op in range(O // OP):
                nc.scalar.dma_start(w_sb[:, :, op * OP:(op + 1) * OP],
                                    wt_r[:, :, op * OP:(op + 1) * OP])
            for i in range(N_WARM):
                wps = psp.tile([P, NT], F32, name=f"warm_{i}", tag="ps")
                nc.tensor.matmul(wps[:], scratch[:, 0:P], scratch[:],
                                 start=True, stop=True)

            # --- Mask broadcast on-chip: ones[128,1] (x) mkrow[1,N] via PE
            # (keeps warming; avoids a 128x replicated mask DMA). ---
            mask_sb = consts.tile([P, N], BF16)
            for n in range(N // NT):
                mps = psp.tile([P, NT], F32, name=f"mps_{n}", tag="ps")
                nc.tensor.matmul(mps[:], ones_sb[:],
                                 mkrow_sb[:, n * NT:(n + 1) * NT],
                                 start=True, stop=True)
                nc.vector.tensor_copy(mask_sb[:, n * NT:(n + 1) * NT], mps[:])

            # --- Main stream: per n-window, 8 serial o-chains of 8 c-steps;
            # 1024-wide windows use a 2-bank psum pair per chain, 512-wide
            # use a single bank. Only 1-2 banks in flight per chain, so
            # evictions stagger and bank reuse has ~4 chains of slack. ---
            n0 = 0
            for ns, nw in enumerate(WINDOWS):
                nsub = nw // NT
                for o in range(OO):
                    pss = [psp.tile([P, NT], F32, name=f"ps_{ns}_{o}_{j}",
                                    tag="ps") for j in range(nsub)]
                    for c in range(CO):
                        w_ap = w_sb[:, c, o * P:(o + 1) * P]
                        for j in range(nsub):
                            nj = n0 + j * NT
                            nc.tensor.matmul(pss[j][:], w_ap,
                                             x_sb[:, c, nj:nj + NT],
                                             start=(c == 0),
                                             stop=(c == CO - 1))
                    ot = outs.tile([P, nw], BF16, name=f"ot_{ns}_{o}",
                                   tag=f"ot{nw}")
                    for j in range(nsub):
                        nj = n0 + j * NT
                        nc.vector.scalar_tensor_tensor(
                            ot[:, j * NT:(j + 1) * NT], pss[j][:],
                            bias_sb[:, o:o + 1], mask_sb[:, nj:nj + NT],
                            op0=mybir.AluOpType.add,
                            op1=mybir.AluOpType.mult)
                    nc.scalar.dma_start(
                        out.ap()[o * P:(o + 1) * P, n0:n0 + nw], ot[:])
                n0 += nw
    nc.compile()
    return nc


_NC_CACHE = None


def _get_module():
    global _NC_CACHE
    if _NC_CACHE is None:
        _NC_CACHE = build_module()
    return _NC_CACHE


def _make_in_maps(cond, x_mask, W, b):
    bf16 = ml_dtypes.bfloat16
    wt = np.ascontiguousarray(
        W.astype(np.float32).sum(axis=0).T.astype(bf16))           # [C, O]
    bv = np.ascontiguousarray(
        b.astype(np.float32).sum(axis=0).reshape(OO, P).T,
        dtype=np.float32)                                          # [128, OO]
    in_maps = []
    for core in range(N_CORES):
        in_maps.append({
            "x": np.ascontiguousarray(cond[core].astype(bf16)),
            "wt": wt,
            "bv": bv,
            "mk": np.ascontiguousarray(x_mask[core, 0].astype(bf16)),
        })
    return in_maps


def run(cond, x_mask, W, b, trace=False, trace_cores=None):
    """Run on hardware; returns (out [B,O,N] fp32, BassKernelResults)."""
    nc = _get_module()
    in_maps = _make_in_maps(cond, x_mask, W, b)
    res = run_bass_kernel_spmd(
        nc, in_maps, core_ids=list(range(N_CORES)),
        trace=trace, trace_cores=trace_cores,
    )
    out = np.stack(
        [res.results[i]["out"].astype(np.float32) for i in range(N_CORES)],
        axis=0)
    return out, res


def kernel(cond, x_mask, W, b):
    out, _ = run(cond, x_mask, W, b)
    return out
